# revision 1
# baseline (speedup 1.0000x reference)
"""ChildSum TreeGRU on 8 Trainium2 NeuronCores.

Data-parallel over trees (16 trees/core). On-device layout is feature-major
([256 feat] -> 2x128 partitions, nodes on the free dim); the host transposes
x's leaf slice in and the output back out. All matmuls run as float32r.

Heap tree, per-core column order is tree-major: col = tree*len + in-level pos.
Levels 10(leaves)..6 are processed per group of 4 trees; level-6 results land
in a joint buffer [128, 16*127] covering heap nodes 0..126 (levels 0..6) for
all 16 trees; levels 5..0 are then processed jointly and the buffer is DMA'd
out in one shot.
"""
import sys

for p in ("/opt/trn_rl_repo", "/root/.axon_site/_ro/trn_rl_repo"):
    if p not in sys.path:
        sys.path.insert(0, p)

import numpy as np
import concourse.tile as tile
from concourse import bacc, mybir
from concourse.bass_utils import run_bass_kernel_spmd

f32 = mybir.dt.float32
f32r = mybir.dt.float32r
AF = mybir.ActivationFunctionType
ALU = mybir.AluOpType

T, DEPTH, NN, H = 128, 11, 2047, 256
NCORES = 8
TPC = T // NCORES          # 16 trees per core
G = 2                      # trees per group
NG = TPC // G              # 4 groups
NLEAF = 1 << (DEPTH - 1)   # 1024
LEAF0 = NLEAF - 1          # 1023
JN = (1 << 7) - 1          # 127 nodes/tree in the joint buffer (levels 0..6)
PS_COLS = 1024             # psum batch (2 banks) consumed by one ACT


def _emit_level(nc, P, tag, NT, Lct, hc3, hc_flat, out3, Wt, bias):
    """One GRU level for NT trees with Lct children per tree.

    hc3:    child-state AP [128, NT, Lct] per half (f32r)
    hc_flat: contiguous 2D view [128, NT*Lct] per half, or None (jbuf)
    out3:   output AP [128, NT, Lpt] per half (f32r)
    """
    Lc = NT * Lct
    Lp = Lc // 2
    uzT, urT, ucT = Wt["uz"], Wt["ur"], Wt["uc"]
    bz, br, bc = bias["bz"], bias["br"], bias["bc"]

    def mm_into(ps, off, lhs, rhs_pair):
        # accumulate both K-halves of one <=512-col chunk into ps[:, off:...]
        n = rhs_pair[0].free_size()
        nc.tensor.matmul(ps[:, off:off + n], lhs[0], rhs_pair[0], start=True, stop=False)
        nc.tensor.matmul(ps[:, off:off + n], lhs[1], rhs_pair[1], start=False, stop=True)

    def child_chunks():
        # yield (cols_off, [rhs_half0, rhs_half1]) chunks of <=512 child cols
        if hc_flat is not None:
            for c0 in range(0, Lc, 512):
                n = min(512, Lc - c0)
                yield c0, [hc_flat[k][:, c0:c0 + n] for k in range(2)]
        else:
            tch = max(1, 512 // Lct)
            for t0 in range(0, NT, tch):
                t1 = min(NT, t0 + tch)
                yield t0 * Lct, [hc3[k][:, t0:t1, :] for k in range(2)]

    # --- h_sum = hc_even + hc_odd (strided), f32r; r-path emitted first so
    # the serial r -> rh -> Uc -> hcand chain starts as early as possible ---
    hs = [P["hs"].tile([128, Lp], f32r, name=f"hs{tag}_{m}", tag=f"hs{m}") for m in range(2)]
    for m in range(2):
        nc.vector.tensor_tensor(hs[m][:], hc3[m][:, :, 0::2], hc3[m][:, :, 1::2], ALU.add)

    # --- r = sigmoid(Ur @ h_sum + br) ---
    r = [P["r"].tile([128, Lp], f32, name=f"r{tag}_{m}", tag=f"r{m}") for m in range(2)]
    for m in range(2):
        lhs = [urT[k][:, m * 128:(m + 1) * 128] for k in range(2)]
        for p0 in range(0, Lp, PS_COLS):
            pn = min(PS_COLS, Lp - p0)
            ps = P["psrc"].tile([128, pn], f32, name=f"psr{tag}_{m}_{p0}", tag="psrc")
            for c0 in range(p0, p0 + pn, 512):
                n = min(512, p0 + pn - c0)
                mm_into(ps, c0 - p0, lhs, [hs[k][:, c0:c0 + n] for k in range(2)])
            nc.scalar.activation(r[m][:, p0:p0 + pn], ps[:], AF.Sigmoid, bias=br[m][:])

    # --- z = sigmoid(Uz @ hc + bz) over all children (fills PE while ACT r runs) ---
    z = [P["z"].tile([128, Lc], f32, name=f"z{tag}_{m}", tag=f"z{m}") for m in range(2)]
    for m in range(2):
        lhs = [uzT[k][:, m * 128:(m + 1) * 128] for k in range(2)]
        for p0 in range(0, Lc, PS_COLS):
            pn = min(PS_COLS, Lc - p0)
            ps = P["psz"].tile([128, pn], f32, name=f"psz{tag}_{m}_{p0}", tag="psz")
            for c0, rhs in child_chunks():
                if p0 <= c0 < p0 + pn:
                    mm_into(ps, c0 - p0, lhs, rhs)
            nc.scalar.activation(z[m][:, p0:p0 + pn], ps[:], AF.Sigmoid, bias=bz[m][:])

    # --- rh = r * h_sum (in place into hs, stays f32r) ---
    for m in range(2):
        nc.vector.tensor_tensor(hs[m][:], r[m][:], hs[m][:], ALU.mult)

    # --- h_cand = tanh(Uc @ rh + bc) ---
    hcand = [P["hc"].tile([128, Lp], f32, name=f"hcand{tag}_{m}", tag=f"hcand{m}") for m in range(2)]
    for m in range(2):
        lhs = [ucT[k][:, m * 128:(m + 1) * 128] for k in range(2)]
        for p0 in range(0, Lp, PS_COLS):
            pn = min(PS_COLS, Lp - p0)
            ps = P["psrc"].tile([128, pn], f32, name=f"psc{tag}_{m}_{p0}", tag="psrc")
            for c0 in range(p0, p0 + pn, 512):
                n = min(512, p0 + pn - c0)
                mm_into(ps, c0 - p0, lhs, [hs[k][:, c0:c0 + n] for k in range(2)])
            nc.scalar.activation(hcand[m][:, p0:p0 + pn], ps[:], AF.Tanh, bias=bc[m][:])

    for m in range(2):
        z3 = z[m][:].rearrange("p (t n) -> p t n", t=NT)
        # zs = z_even + z_odd  (before z is overwritten by zh); reuses the r slot
        zs = P["r"].tile([128, Lp], f32, name=f"zs{tag}_{m}", tag=f"r{m}")
        nc.vector.tensor_tensor(zs[:], z3[:, :, 0::2], z3[:, :, 1::2], ALU.add)
        # zh = z * hc, in place into z (DVE: gpsimd would contend for the
        # shared DVE/GpSimd SBUF port pair and slow both engines ~4x)
        nc.vector.tensor_tensor(z[m][:], z[m][:], hc3[m].bitcast(f32), ALU.mult)
        # zh_sum = zh_even + zh_odd; reuses the h_sum slot
        zhs = P["hs"].tile([128, Lp], f32, name=f"zhs{tag}_{m}", tag=f"hs{m}")
        nc.vector.tensor_tensor(zhs[:], z3[:, :, 0::2], z3[:, :, 1::2], ALU.add)
        # t = (zs - 1) * h_cand, in place into hcand
        nc.vector.scalar_tensor_tensor(hcand[m][:], zs[:], 1.0, hcand[m][:], ALU.subtract, ALU.mult)
        # h_new = zh_sum - t  -> out3 (f32r)
        nc.vector.tensor_tensor(out3[m], zhs[:], hcand[m][:], ALU.subtract)


def _build():
    nc = bacc.Bacc("TRN2", debug=False)

    xT_d = nc.dram_tensor("xT", [H, TPC * NLEAF], f32r, kind="ExternalInput")
    wT_d = nc.dram_tensor("wT", [H, H], f32r, kind="ExternalInput")
    uzT_d = nc.dram_tensor("uzT", [H, H], f32r, kind="ExternalInput")
    urT_d = nc.dram_tensor("urT", [H, H], f32r, kind="ExternalInput")
    ucT_d = nc.dram_tensor("ucT", [H, H], f32r, kind="ExternalInput")
    bw_d = nc.dram_tensor("bw", [H, 1], f32, kind="ExternalInput")
    bz_d = nc.dram_tensor("bz", [H, 1], f32, kind="ExternalInput")
    br_d = nc.dram_tensor("br", [H, 1], f32, kind="ExternalInput")
    bc_d = nc.dram_tensor("bc", [H, 1], f32, kind="ExternalInput")
    hout_d = nc.dram_tensor("h_out", [H, TPC, NN], f32, kind="ExternalOutput")

    with tile.TileContext(nc) as tc:
        from contextlib import ExitStack
        with ExitStack() as ctx:
            P = {}
            P["const"] = ctx.enter_context(tc.tile_pool(name="const", bufs=1))
            P["xg"] = ctx.enter_context(tc.tile_pool(name="xg", bufs=2))
            P["h10"] = ctx.enter_context(tc.tile_pool(name="h10", bufs=2))
            P["hl"] = ctx.enter_context(tc.tile_pool(name="hl", bufs=2))
            P["jbuf"] = ctx.enter_context(tc.tile_pool(name="jbuf", bufs=1))
            P["z"] = ctx.enter_context(tc.tile_pool(name="z", bufs=2))
            P["hs"] = ctx.enter_context(tc.tile_pool(name="hs", bufs=2))
            P["r"] = ctx.enter_context(tc.tile_pool(name="r", bufs=2))
            P["hc"] = ctx.enter_context(tc.tile_pool(name="hc", bufs=2))
            P["psz"] = ctx.enter_context(tc.tile_pool(name="psz", bufs=2, space="PSUM"))
            P["psrc"] = ctx.enter_context(tc.tile_pool(name="psrc", bufs=2, space="PSUM"))

            cp = P["const"]
            Wt = {}
            for nm, d in (("w", wT_d), ("uz", uzT_d), ("ur", urT_d), ("uc", ucT_d)):
                Wt[nm] = [cp.tile([128, H], f32r, name=f"{nm}T{k}") for k in range(2)]
                for k in range(2):
                    nc.sync.dma_start(Wt[nm][k][:], d.ap()[k * 128:(k + 1) * 128, :])
            bias = {}
            for nm, d in (("bw", bw_d), ("bz", bz_d), ("br", br_d), ("bc", bc_d)):
                bias[nm] = [cp.tile([128, 1], f32, name=f"{nm}{m}") for m in range(2)]
                for m in range(2):
                    nc.sync.dma_start(bias[nm][m][:], d.ap()[m * 128:(m + 1) * 128, :])

            # joint buffer: heap nodes 0..126 for all 16 trees, per half
            jbuf = [P["jbuf"].tile([128, TPC * JN], f32r, name=f"jbuf{m}") for m in range(2)]
            jv = [jbuf[m][:].rearrange("p (t n) -> p t n", t=TPC) for m in range(2)]

            def emit_leaf(g):
                gt = f"g{g}"
                xg = [P["xg"].tile([128, G * NLEAF], f32r, name=f"x{gt}_{k}", tag="xg")
                      for k in range(2)]
                for k in range(2):
                    for piece in range(0, G * NLEAF, 1024):
                        pend = min(piece + 1024, G * NLEAF)
                        nc.sync.dma_start(
                            xg[k][:, piece:pend],
                            xT_d.ap()[k * 128:(k + 1) * 128,
                                      g * G * NLEAF + piece:g * G * NLEAF + pend])
                h10 = [P["h10"].tile([128, G * NLEAF], f32r, name=f"h10{gt}_{m}", tag=f"h10{m}")
                       for m in range(2)]
                for m in range(2):
                    lhs = [Wt["w"][k][:, m * 128:(m + 1) * 128] for k in range(2)]
                    for p0 in range(0, G * NLEAF, PS_COLS):
                        pn = min(PS_COLS, G * NLEAF - p0)
                        ps = P["psz"].tile([128, pn], f32, name=f"psx{gt}_{m}_{p0}", tag="psz")
                        for c0 in range(p0, p0 + pn, 512):
                            n = min(512, p0 + pn - c0)
                            nc.tensor.matmul(ps[:, c0 - p0:c0 - p0 + n], lhs[0],
                                             xg[0][:, c0:c0 + n], start=True, stop=False)
                            nc.tensor.matmul(ps[:, c0 - p0:c0 - p0 + n], lhs[1],
                                             xg[1][:, c0:c0 + n], start=False, stop=True)
                        nc.scalar.activation(h10[m][:, p0:p0 + pn], ps[:], AF.Tanh,
                                             bias=bias["bw"][m][:])
                    nc.sync.dma_start(
                        hout_d.ap()[m * 128:(m + 1) * 128, g * G:(g + 1) * G,
                                    LEAF0:LEAF0 + NLEAF],
                        h10[m][:].rearrange("p (t n) -> p t n", t=G).bitcast(f32))
                return h10

            def emit_lvl(g, lv, hchild):
                gt = f"g{g}"
                Lct = 2 ** (lv + 1)
                Lpt = 2 ** lv
                hc3 = [hchild[m][:].rearrange("p (t n) -> p t n", t=G) for m in range(2)]
                hc_flat = [hchild[m][:] for m in range(2)]
                if lv == 6:
                    out3 = [jv[m][:, g * G:(g + 1) * G, Lpt - 1:2 * Lpt - 1]
                            for m in range(2)]
                    hnew = None
                else:
                    hnew = [P["hl"].tile([128, G * Lpt], f32r,
                                         name=f"h{lv}{gt}_{m}", tag=f"h{lv}_{m}")
                            for m in range(2)]
                    out3 = [hnew[m][:].rearrange("p (t n) -> p t n", t=G)
                            for m in range(2)]
                _emit_level(nc, P, f"{gt}l{lv}", G, Lct, hc3, hc_flat, out3, Wt, bias)
                if lv > 6:
                    for m in range(2):
                        nc.sync.dma_start(
                            hout_d.ap()[m * 128:(m + 1) * 128, g * G:(g + 1) * G,
                                        Lpt - 1:2 * Lpt - 1],
                            hnew[m][:].rearrange("p (t n) -> p t n", t=G).bitcast(f32))
                return hnew

            # wavefront: stage s of group g is emitted at tick t = g + s
            # (stage 0 = leaf, stages 1..4 = levels 9..6) so PE always has a
            # dense leaf/z matmul stream while DVE/ACT work the gate math
            gstate = {}
            for t in range(NG + 4):
                for g in range(NG):
                    s = t - g
                    if s < 0 or s > 4:
                        continue
                    if s == 0:
                        gstate[g] = emit_leaf(g)
                    else:
                        gstate[g] = emit_lvl(g, 10 - s, gstate[g])

            # l6 region of the joint buffer is complete: stream it out
            for m in range(2):
                nc.sync.dma_start(
                    hout_d.ap()[m * 128:(m + 1) * 128, :, 63:JN],
                    jv[m][:, :, 63:JN].bitcast(f32))

            # ---- joint levels 5..0 over jbuf, streaming each level out ----
            for m in range(2):
                nc.sync.dma_start(
                    hout_d.ap()[m * 128:(m + 1) * 128, :, 63:JN],
                    jv[m][:, :, 63:JN].bitcast(f32))
            for lv in range(5, -1, -1):
                Lct = 2 ** (lv + 1)
                Lpt = 2 ** lv
                hc3 = [jv[m][:, :, Lct - 1:2 * Lct - 1] for m in range(2)]
                out3 = [jv[m][:, :, Lpt - 1:2 * Lpt - 1] for m in range(2)]
                _emit_level(nc, P, f"j{lv}", TPC, Lct, hc3, None, out3, Wt, bias)
                for m in range(2):
                    nc.sync.dma_start(
                        hout_d.ap()[m * 128:(m + 1) * 128, :, Lpt - 1:2 * Lpt - 1],
                        jv[m][:, :, Lpt - 1:2 * Lpt - 1].bitcast(f32))

    nc.compile()
    return nc


_NC = None


def _get_nc():
    global _NC
    if _NC is None:
        _NC = _build()
    return _NC


def make_in_maps(inputs):
    x = np.asarray(inputs["x"], np.float32)
    W = np.asarray(inputs["W"], np.float32)
    bW = np.asarray(inputs["bW"], np.float32).reshape(H, 1)
    Ur = np.asarray(inputs["Ur"], np.float32)
    br = np.asarray(inputs["br"], np.float32).reshape(H, 1)
    Uc = np.asarray(inputs["Uc"], np.float32)
    bc = np.asarray(inputs["bc"], np.float32).reshape(H, 1)
    Uz = np.asarray(inputs["Uz"], np.float32)
    bz = np.asarray(inputs["bz"], np.float32).reshape(H, 1)
    shared = {
        "wT": np.ascontiguousarray(W.T), "uzT": np.ascontiguousarray(Uz.T),
        "urT": np.ascontiguousarray(Ur.T), "ucT": np.ascontiguousarray(Uc.T),
        "bw": bW, "bz": bz, "br": br, "bc": bc,
    }
    in_maps = []
    for c in range(NCORES):
        xs = x[c * TPC:(c + 1) * TPC, LEAF0:, :]          # [16, 1024, 256]
        xTc = np.ascontiguousarray(xs.transpose(2, 0, 1)).reshape(H, TPC * NLEAF)
        in_maps.append({"xT": xTc, **shared})
    return in_maps


def assemble_out(core_outs):
    out = np.empty((T, NN, H), np.float32)
    for c in range(NCORES):
        # [256, 16, 2047] -> [16, 2047, 256]
        out[c * TPC:(c + 1) * TPC] = core_outs[c].transpose(1, 2, 0)
    return out


def kernel(**inputs):
    nc = _get_nc()
    in_maps = make_in_maps(inputs)
    res = run_bass_kernel_spmd(nc, in_maps, list(range(NCORES)))
    return assemble_out([r["h_out"] for r in res.results])



# revision 10
# speedup vs baseline: 1.2148x; 1.2148x over previous
"""ChildSum TreeGRU on 8 Trainium2 NeuronCores.

Data-parallel over trees (16 trees/core). Feature-major on device: a value
tile is [128 partitions, 2, cols] fp16 where axis1 is the feature half
(m=0 -> features 0..127, m=1 -> 128..255); partitions carry features, cols
carry (tree, node). All matmuls are fp16 with fp32 PSUM accumulation.

Within every level block, nodes are stored parity-major ([children with even
in-level index | odd index]) so the child-pair sums (h_sum, z_sum, zh_sum)
are contiguous block adds on DVE (2x fp16 mode). The host pre-permutes the
leaf columns of x and un-permutes the output, so the device never shuffles.

DRAM output is a flat [256, 32752] fp16 buffer laid out exactly like SBUF
(big contiguous DMA runs); the host gathers it back to [T, N, H] f32.
"""
import sys

for p in ("/opt/trn_rl_repo", "/root/.axon_site/_ro/trn_rl_repo"):
    if p not in sys.path:
        sys.path.insert(0, p)

import numpy as np
import concourse.tile as tile
from concourse import bacc, mybir
from concourse.bass_utils import run_bass_kernel_spmd

f16 = mybir.dt.float16
f32 = mybir.dt.float32
AF = mybir.ActivationFunctionType
ALU = mybir.AluOpType

T, DEPTH, NN, H = 128, 11, 2047, 256
NCORES = 8
TPC = T // NCORES          # 16 trees per core
G = 2                      # trees per group
NG = TPC // G              # 8 groups
NLEAF = 1 << (DEPTH - 1)   # 1024
LEAF0 = NLEAF - 1          # 1023
JN = (1 << 7) - 1          # 127 nodes/tree in the joint buffer (levels 0..6)

# device DRAM output column offsets (per core, 16 trees x 2047 nodes)
OFF_LF = 0                  # leaves: g*2048 + tg*1024 + pos
OFF_L9 = 16384              # g*1024 + tg*512 + pos
OFF_L8 = 24576              # g*512 + tg*256 + pos
OFF_L7 = 28672              # g*256 + tg*128 + pos
OFF_JB = 30720              # t*127 + (2^lv-1) + pos, lv<=6
NCOLS = 32752


def _mm_batch(nc, ps, n, lhs_m, rhs_k):
    """Accumulate one GEMM chunk into psum tile ps (n <= 1024 cols).

    lhs_m: [lhsT_k0, lhsT_k1] each [128,128]; rhs_k(k, c0, cn) -> AP of cn
    cols. Weight-stationary: all chunks of k0 first, then k1.
    """
    for k in range(2):
        for c0 in range(0, n, 512):
            cn = min(512, n - c0)
            nc.tensor.matmul(ps[:, c0:c0 + cn], lhs_m[k], rhs_k(k, c0, cn),
                             start=(k == 0), stop=(k == 1))


def _gemm_act(nc, P, pool, tag, Wkey, Wt, bias_key, bias, af, rhs_k_fn, out_mc, Lc):
    """out[:, m, :] = af(W @ rhs + b) for both halves, 1024-col psum chunks.

    rhs_k_fn(k, c0, cn) -> AP; out_mc(m, c0, cn) -> SBUF dest AP.
    """
    for m in range(2):
        lhs_m = [Wt[Wkey][k][:, m * 128:(m + 1) * 128] for k in range(2)]
        for p0 in range(0, Lc, 1024):
            pn = min(1024, Lc - p0)
            ps = P[pool].tile([128, pn], f32, name=f"ps{tag}_{m}_{p0}", tag=pool)
            _mm_batch(nc, ps, pn, lhs_m,
                      lambda k, c0, cn: rhs_k_fn(k, p0 + c0, cn))
            nc.scalar.activation(out_mc(m, p0, pn), ps[:], af,
                                 bias=bias[bias_key][m][:])


def _emit_level(nc, P, tag, NT, Lct, hc4, out_scat, Wt, bias, lv):
    """One GRU level for NT trees, Lct children/tree (parity-major blocks).

    hc4: child AP [128, 2, NT, Lct] fp16 (axis1 = feature half)
    out_scat(m) -> dest AP for the final combine, parity-scattered
                   ([128, NT, Lpt/2, 2]) or plain [128, NT, 1] at the root.
    """
    Lpt = Lct // 2            # parents per tree
    Lc = NT * Lct             # child cols per half
    Lp = NT * Lpt

    def half(t2, C, m):       # [128, C] slice of a 2D [128, 2*C] tile
        return t2[:, m * C:(m + 1) * C]

    def h3(t2, C, m):         # [128, NT, c] view of one half
        return half(t2, C, m).rearrange("p (t j) -> p t j", t=NT)

    def hc_eo(m, par):
        v = hc4[:, m].rearrange("p t (par j) -> p t par j", par=2)
        return v[:, :, par, :]                      # [128, NT, Lct/2]

    # --- h_sum = hc_even + hc_odd (contiguous 2x fp16) ---
    hs = P["hs"].tile([128, 2 * Lp], f16, name=f"hs{tag}", tag="hs")
    for m in range(2):
        nc.vector.tensor_tensor(h3(hs, Lp, m), hc_eo(m, 0), hc_eo(m, 1), ALU.add)

    # --- r = sigmoid(Ur @ h_sum + br); serial chain starts first ---
    r = P["r"].tile([128, 2 * Lp], f16, name=f"r{tag}", tag="r")
    _gemm_act(nc, P, "psrc", f"r{tag}", "ur", Wt, "br", bias, AF.Sigmoid,
              lambda k, c0, cn: half(hs, Lp, k)[:, c0:c0 + cn],
              lambda m, c0, cn: half(r, Lp, m)[:, c0:c0 + cn], Lp)

    # --- z = sigmoid(Uz @ hc + bz) (fills PE while r's ACT runs) ---
    z = P["z"].tile([128, 2 * Lc], f16, name=f"z{tag}", tag="z")

    jointbuf = lv <= 5                # hc4 is a strided jbuf view

    def z_rhs(k, c0, cn):
        v = hc4[:, k]                                # [128, NT, Lct]
        if jointbuf:
            assert c0 % Lct == 0 and cn % Lct == 0
            return v[:, c0 // Lct:(c0 + cn) // Lct, :]
        return v.rearrange("p t j -> p (t j)")[:, c0:c0 + cn]

    _gemm_act(nc, P, "psz", f"z{tag}", "uz", Wt, "bz", bias, AF.Sigmoid,
              z_rhs, lambda m, c0, cn: half(z, Lc, m)[:, c0:c0 + cn], Lc)

    # --- zs = z_even + z_odd (before z is overwritten by z*hc) ---
    zs = P["zs"].tile([128, 2 * Lp], f16, name=f"zs{tag}", tag="zs")
    for m in range(2):
        zv = half(z, Lc, m).rearrange("p (t par j) -> p t par j", t=NT, par=2)
        nc.vector.tensor_tensor(h3(zs, Lp, m), zv[:, :, 0, :], zv[:, :, 1, :],
                                ALU.add)

    # --- rh = r * h_sum (in place into hs) ---
    nc.vector.tensor_tensor(hs[:], r[:], hs[:], ALU.mult)

    # --- h_cand = tanh(Uc @ rh + bc) ---
    hcand = P["hc"].tile([128, 2 * Lp], f16, name=f"hcand{tag}", tag="hc")
    _gemm_act(nc, P, "psrc", f"c{tag}", "uc", Wt, "bc", bias, AF.Tanh,
              lambda k, c0, cn: half(hs, Lp, k)[:, c0:c0 + cn],
              lambda m, c0, cn: half(hcand, Lp, m)[:, c0:c0 + cn], Lp)

    # --- zh = z * hc (in place into z, 4D to handle strided jbuf hc) ---
    z4 = z[:].rearrange("p (m t j) -> p m t j", m=2, t=NT)
    for m in range(2):
        nc.vector.tensor_tensor(z4[:, m], z4[:, m], hc4[:, m], ALU.mult)

    # --- zh_sum = zh_even + zh_odd ---
    zhs = P["zhs"].tile([128, 2 * Lp], f16, name=f"zhs{tag}", tag="zhs")
    for m in range(2):
        zv = half(z, Lc, m).rearrange("p (t par j) -> p t par j", t=NT, par=2)
        nc.vector.tensor_tensor(h3(zhs, Lp, m), zv[:, :, 0, :], zv[:, :, 1, :],
                                ALU.add)

    # --- t = (zs - 1) * h_cand, in place; h_new = zh_sum - t (scattered) ---
    nc.vector.scalar_tensor_tensor(hcand[:], zs[:], 1.0, hcand[:],
                                   ALU.subtract, ALU.mult)
    for m in range(2):
        if lv == 0:
            src_z, src_c = h3(zhs, Lp, m), h3(hcand, Lp, m)
        else:
            src_z = half(zhs, Lp, m).rearrange("p (t j par) -> p t j par",
                                           t=NT, par=2)
            src_c = half(hcand, Lp, m).rearrange("p (t j par) -> p t j par",
                                             t=NT, par=2)
        nc.vector.tensor_tensor(out_scat(m), src_z, src_c, ALU.subtract)


def _build():
    nc = bacc.Bacc("TRN2", debug=False)

    xT_d = nc.dram_tensor("xT", [H, TPC * NLEAF], f16, kind="ExternalInput")
    wT_d = nc.dram_tensor("wT", [H, H], f16, kind="ExternalInput")
    uzT_d = nc.dram_tensor("uzT", [H, H], f16, kind="ExternalInput")
    urT_d = nc.dram_tensor("urT", [H, H], f16, kind="ExternalInput")
    ucT_d = nc.dram_tensor("ucT", [H, H], f16, kind="ExternalInput")
    bw_d = nc.dram_tensor("bw", [H, 1], f32, kind="ExternalInput")
    bz_d = nc.dram_tensor("bz", [H, 1], f32, kind="ExternalInput")
    br_d = nc.dram_tensor("br", [H, 1], f32, kind="ExternalInput")
    bc_d = nc.dram_tensor("bc", [H, 1], f32, kind="ExternalInput")
    hout_d = nc.dram_tensor("h_out", [H, NCOLS], f16, kind="ExternalOutput")

    def dview(c0, n):
        # DRAM dest [128, 2, n] view of hout cols [c0, c0+n)
        return hout_d.ap()[:, c0:c0 + n].rearrange("(m p) c -> p m c", m=2)

    with tile.TileContext(nc) as tc:
        from contextlib import ExitStack
        with ExitStack() as ctx:
            P = {}
            for nm, bufs in (("const", 1), ("xg", 2), ("h10", 2), ("hl", 2),
                             ("jbuf", 1), ("z", 2), ("hs", 2), ("zs", 2),
                             ("zhs", 2), ("r", 2), ("hc", 2)):
                P[nm] = ctx.enter_context(tc.tile_pool(name=nm, bufs=bufs))
            P["psz"] = ctx.enter_context(tc.tile_pool(name="psz", bufs=2, space="PSUM"))
            P["psrc"] = ctx.enter_context(tc.tile_pool(name="psrc", bufs=2, space="PSUM"))

            cp = P["const"]
            Wt = {}
            for nm, d in (("w", wT_d), ("uz", uzT_d), ("ur", urT_d), ("uc", ucT_d)):
                Wt[nm] = [cp.tile([128, H], f16, name=f"{nm}T{k}") for k in range(2)]
                for k in range(2):
                    nc.sync.dma_start(Wt[nm][k][:], d.ap()[k * 128:(k + 1) * 128, :])
            bias = {}
            for nm, d in (("bw", bw_d), ("bz", bz_d), ("br", br_d), ("bc", bc_d)):
                bias[nm] = [cp.tile([128, 1], f32, name=f"{nm}{m}") for m in range(2)]
                for m in range(2):
                    nc.sync.dma_start(bias[nm][m][:], d.ap()[m * 128:(m + 1) * 128, :])

            # joint buffer: heap nodes 0..126 per tree (levels 0..6)
            jbuf = P["jbuf"].tile([128, 2 * TPC * JN], f16, name="jbuf")
            jv4 = jbuf[:].rearrange("p (m t n) -> p m t n", m=2, t=TPC)

            def emit_leaf(g):
                gt = f"g{g}"
                C = G * NLEAF
                xg = P["xg"].tile([128, 2 * C], f16, name=f"x{gt}", tag="xg")
                for k in range(2):
                    for c0 in range(0, C, 1024):
                        nc.sync.dma_start(
                            xg[:, k * C + c0:k * C + c0 + 1024],
                            xT_d.ap()[k * 128:(k + 1) * 128,
                                      g * C + c0:g * C + c0 + 1024])
                h10 = P["h10"].tile([128, 2 * C], f16, name=f"h10{gt}", tag="h10")
                _gemm_act(nc, P, "psz", f"x{gt}", "w", Wt, "bw", bias, AF.Tanh,
                          lambda k, c0, cn: xg[:, k * C + c0:k * C + c0 + cn],
                          lambda m, c0, cn: h10[:, m * C + c0:m * C + c0 + cn],
                          C)
                nc.sync.dma_start(
                    dview(OFF_LF + g * C, C),
                    h10[:].rearrange("p (m c) -> p m c", m=2))
                return h10

            def emit_lvl(g, lv, hchild):
                gt = f"g{g}"
                Lct = 1 << (lv + 1)        # children per tree (level lv+1 size)
                Lpt = 1 << lv
                hc4 = hchild[:].rearrange("p (m t j) -> p m t j", m=2, t=G)
                if lv == 6:
                    def out_scat(m):
                        blk = jv4[:, m, g * G:(g + 1) * G, 63:127]
                        return blk.rearrange("p t (par j) -> p t j par", par=2)
                    _emit_level(nc, P, f"{gt}l{lv}", G, Lct, hc4, out_scat,
                                Wt, bias, lv)
                    return None
                hnew = P["hl"].tile([128, 2 * G * Lpt], f16,
                                    name=f"h{lv}{gt}", tag=f"h{lv}")

                def out_scat(m):
                    v = hnew[:, m * G * Lpt:(m + 1) * G * Lpt]
                    return v.rearrange("p (t par j) -> p t j par", t=G, par=2)
                _emit_level(nc, P, f"{gt}l{lv}", G, Lct, hc4, out_scat,
                            Wt, bias, lv)
                off = {9: OFF_L9, 8: OFF_L8, 7: OFF_L7}[lv]
                nc.sync.dma_start(
                    dview(off + g * G * Lpt, G * Lpt),
                    hnew[:].rearrange("p (m c) -> p m c", m=2))
                return hnew

            # wavefront: stage s of group g emitted at tick t = g + s
            gstate = {}
            for t in range(NG + 4):
                for g in range(NG):
                    s = t - g
                    if s < 0 or s > 4:
                        continue
                    if s == 0:
                        gstate[g] = emit_leaf(g)
                    else:
                        gstate[g] = emit_lvl(g, 10 - s, gstate[g])

            # joint levels 5..0 over jbuf
            for lv in range(5, -1, -1):
                Lct = 1 << (lv + 1)
                Lpt = 1 << lv
                hc4 = jv4[:, :, :, Lct - 1:2 * Lct - 1]
                if lv == 0:
                    def out_scat(m):
                        return jv4[:, m, :, 0:1]
                else:
                    def out_scat(m):
                        blk = jv4[:, m, :, Lpt - 1:2 * Lpt - 1]
                        return blk.rearrange("p t (par j) -> p t j par", par=2)
                _emit_level(nc, P, f"j{lv}", TPC, Lct, hc4, out_scat,
                            Wt, bias, lv)

            nc.sync.dma_start(dview(OFF_JB, TPC * JN),
                              jbuf[:].rearrange("p (m c) -> p m c", m=2))

    nc.compile()
    return nc


_NC = None


def _get_nc():
    global _NC
    if _NC is None:
        _NC = _build()
    return _NC


# position perm: parity-major order within a level block
_POS_LEAF = np.concatenate([np.arange(0, NLEAF, 2), np.arange(1, NLEAF, 2)])


def make_in_maps(inputs):
    x = np.asarray(inputs["x"], np.float32)
    W = np.asarray(inputs["W"], np.float32)
    bW = np.asarray(inputs["bW"], np.float32).reshape(H, 1)
    Ur = np.asarray(inputs["Ur"], np.float32)
    br = np.asarray(inputs["br"], np.float32).reshape(H, 1)
    Uc = np.asarray(inputs["Uc"], np.float32)
    bc = np.asarray(inputs["bc"], np.float32).reshape(H, 1)
    Uz = np.asarray(inputs["Uz"], np.float32)
    bz = np.asarray(inputs["bz"], np.float32).reshape(H, 1)
    shared = {
        "wT": np.ascontiguousarray(W.T).astype(np.float16),
        "uzT": np.ascontiguousarray(Uz.T).astype(np.float16),
        "urT": np.ascontiguousarray(Ur.T).astype(np.float16),
        "ucT": np.ascontiguousarray(Uc.T).astype(np.float16),
        "bw": bW, "bz": bz, "br": br, "bc": bc,
    }
    in_maps = []
    for c in range(NCORES):
        # leaf slice, parity-major leaf order: [16, 1024, 256]
        xs = x[c * TPC:(c + 1) * TPC, LEAF0:, :][:, _POS_LEAF, :]
        xTc = np.ascontiguousarray(
            xs.transpose(2, 0, 1).reshape(H, TPC * NLEAF)).astype(np.float16)
        in_maps.append({"xT": xTc, **shared})
    return in_maps


def _out_perm():
    """perm[t, n] -> device column of (tree t, heap node n)."""
    perm = np.empty((TPC, NN), np.int64)
    for n in range(NN):
        lam = (n + 1).bit_length() - 1
        i = n - ((1 << lam) - 1)
        L = 1 << lam
        pos = i if L == 1 else (i & 1) * (L // 2) + (i >> 1)
        for t in range(TPC):
            g, tg = t // G, t % G
            if lam == 10:
                col = OFF_LF + g * (G * 1024) + tg * 1024 + pos
            elif lam == 9:
                col = OFF_L9 + g * (G * 512) + tg * 512 + pos
            elif lam == 8:
                col = OFF_L8 + g * (G * 256) + tg * 256 + pos
            elif lam == 7:
                col = OFF_L7 + g * (G * 128) + tg * 128 + pos
            else:
                col = OFF_JB + t * JN + (L - 1) + pos
            perm[t, n] = col
    return perm


_PERM = _out_perm()


def assemble_out(core_outs):
    out = np.empty((T, NN, H), np.float32)
    for c in range(NCORES):
        buf = core_outs[c]                       # [256, 32752] f16
        g = buf[:, _PERM.ravel()]                # [256, 16*2047]
        out[c * TPC:(c + 1) * TPC] = (
            g.reshape(H, TPC, NN).transpose(1, 2, 0).astype(np.float32))
    return out


def kernel(**inputs):
    nc = _get_nc()
    in_maps = make_in_maps(inputs)
    res = run_bass_kernel_spmd(nc, in_maps, list(range(NCORES)))
    return assemble_out([r["h_out"] for r in res.results])


# revision 11
# speedup vs baseline: 1.5663x; 1.2894x over previous
"""ChildSum TreeGRU on 8 Trainium2 NeuronCores.

Data-parallel over trees (16 trees/core). Feature-major on device: a value
tile is [128 partitions, 2, cols] fp16 where axis1 is the feature half
(m=0 -> features 0..127, m=1 -> 128..255); partitions carry features, cols
carry (tree, node). All matmuls are fp16 with fp32 PSUM accumulation.

Within every level block, nodes are stored parity-major ([children with even
in-level index | odd index]) so the child-pair sums (h_sum, z_sum, zh_sum)
are contiguous block adds on DVE (2x fp16 mode). The host pre-permutes the
leaf columns of x and un-permutes the output, so the device never shuffles.

DRAM output is a flat [256, 32752] fp16 buffer laid out exactly like SBUF
(big contiguous DMA runs); the host gathers it back to [T, N, H] f32.
"""
import sys

for p in ("/opt/trn_rl_repo", "/root/.axon_site/_ro/trn_rl_repo"):
    if p not in sys.path:
        sys.path.insert(0, p)

import numpy as np
import concourse.tile as tile
from concourse import bacc, mybir
from concourse.bass_utils import run_bass_kernel_spmd

f16 = mybir.dt.float16
f32 = mybir.dt.float32
AF = mybir.ActivationFunctionType
ALU = mybir.AluOpType

T, DEPTH, NN, H = 128, 11, 2047, 256
NCORES = 8
TPC = T // NCORES          # 16 trees per core
G = 2                      # trees per group
NG = TPC // G              # 8 groups
NLEAF = 1 << (DEPTH - 1)   # 1024
LEAF0 = NLEAF - 1          # 1023
JN = (1 << 7) - 1          # 127 nodes/tree in the joint buffer (levels 0..6)

# device DRAM output column offsets (per core, 16 trees x 2047 nodes)
OFF_LF = 0                  # leaves: g*2048 + tg*1024 + pos
OFF_L9 = 16384              # g*1024 + tg*512 + pos
OFF_L8 = 24576              # g*512 + tg*256 + pos
OFF_L7 = 28672              # g*256 + tg*128 + pos
OFF_JB = 30720              # t*127 + (2^lv-1) + pos, lv<=6
NCOLS = 32752


def _mm_batch(nc, ps, n, lhs_m, rhs_k):
    """Accumulate one GEMM chunk into psum tile ps (n <= 1024 cols).

    lhs_m: [lhsT_k0, lhsT_k1] each [128,128]; rhs_k(k, c0, cn) -> AP of cn
    cols. Weight-stationary: all chunks of k0 first, then k1.
    """
    for k in range(2):
        for c0 in range(0, n, 512):
            cn = min(512, n - c0)
            nc.tensor.matmul(ps[:, c0:c0 + cn], lhs_m[k], rhs_k(k, c0, cn),
                             start=(k == 0), stop=(k == 1))


def _gemm_act(nc, P, pool, tag, Wkey, Wt, bias_key, bias, af, rhs_k_fn, out_mc, Lc):
    """out[:, m, :] = af(W @ rhs + b) for both halves, 1024-col psum chunks.

    rhs_k_fn(k, c0, cn) -> AP; out_mc(m, c0, cn) -> SBUF dest AP.
    """
    for m in range(2):
        lhs_m = [Wt[Wkey][k][:, m * 128:(m + 1) * 128] for k in range(2)]
        for p0 in range(0, Lc, 1024):
            pn = min(1024, Lc - p0)
            ps = P[pool].tile([128, pn], f32, name=f"ps{tag}_{m}_{p0}", tag=pool)
            _mm_batch(nc, ps, pn, lhs_m,
                      lambda k, c0, cn: rhs_k_fn(k, p0 + c0, cn))
            nc.scalar.activation(out_mc(m, p0, pn), ps[:], af,
                                 bias=bias[bias_key][m][:])


def _emit_level(nc, P, tag, NT, Lct, hc4, out_scat, Wt, bias, lv, out_half=None):
    """One GRU level for NT trees, Lct children/tree (parity-major blocks).

    hc4: child AP [128, 2, NT, Lct] fp16 (axis1 = feature half)
    out_scat(m) -> dest AP for the final combine, parity-scattered
                   ([128, NT, Lpt/2, 2]) or plain [128, NT, 1] at the root.
    """
    Lpt = Lct // 2            # parents per tree
    Lc = NT * Lct             # child cols per half
    Lp = NT * Lpt

    def half(t2, C, m):       # [128, C] slice of a 2D [128, 2*C] tile
        return t2[:, m * C:(m + 1) * C]

    def h3(t2, C, m):         # [128, NT, c] view of one half
        return half(t2, C, m).rearrange("p (t j) -> p t j", t=NT)

    def hc_eo(m, par):
        v = hc4[:, m].rearrange("p t (par j) -> p t par j", par=2)
        return v[:, :, par, :]                      # [128, NT, Lct/2]

    # --- h_sum = hc_even + hc_odd (contiguous 2x fp16) ---
    wavef = lv > 5                    # hc4/tiles are contiguous 2D layouts
    hs = P["hs"].tile([128, 2 * Lp], f16, name=f"hs{tag}", tag="hs")
    for m in range(2):
        if wavef:
            for t in range(NT):
                hct = hc4[:, m, t, :]
                nc.vector.tensor_tensor(
                    half(hs, Lp, m)[:, t * Lpt:(t + 1) * Lpt],
                    hct[:, 0:Lpt], hct[:, Lpt:Lct], ALU.add)
        else:
            nc.vector.tensor_tensor(h3(hs, Lp, m), hc_eo(m, 0), hc_eo(m, 1),
                                    ALU.add)

    # --- r = sigmoid(Ur @ h_sum + br); serial chain starts first ---
    r = P["r"].tile([128, 2 * Lp], f16, name=f"r{tag}", tag="r")
    _gemm_act(nc, P, "psrc", f"r{tag}", "ur", Wt, "br", bias, AF.Sigmoid,
              lambda k, c0, cn: half(hs, Lp, k)[:, c0:c0 + cn],
              lambda m, c0, cn: half(r, Lp, m)[:, c0:c0 + cn], Lp)

    # --- z = sigmoid(Uz @ hc + bz) (fills PE while r's ACT runs) ---
    z = P["z"].tile([128, 2 * Lc], f16, name=f"z{tag}", tag="z")

    jointbuf = lv <= 5                # hc4 is a strided jbuf view

    def z_rhs(k, c0, cn):
        v = hc4[:, k]                                # [128, NT, Lct]
        if jointbuf:
            assert c0 % Lct == 0 and cn % Lct == 0
            return v[:, c0 // Lct:(c0 + cn) // Lct, :]
        return v.rearrange("p t j -> p (t j)")[:, c0:c0 + cn]

    _gemm_act(nc, P, "psz", f"z{tag}", "uz", Wt, "bz", bias, AF.Sigmoid,
              z_rhs, lambda m, c0, cn: half(z, Lc, m)[:, c0:c0 + cn], Lc)

    # --- zs = z_even + z_odd (before z is overwritten by z*hc) ---
    zs = P["zs"].tile([128, 2 * Lp], f16, name=f"zs{tag}", tag="zs")
    for m in range(2):
        if wavef:
            for t in range(NT):
                zt = half(z, Lc, m)[:, t * Lct:(t + 1) * Lct]
                nc.vector.tensor_tensor(
                    half(zs, Lp, m)[:, t * Lpt:(t + 1) * Lpt],
                    zt[:, 0:Lpt], zt[:, Lpt:Lct], ALU.add)
        else:
            zv = half(z, Lc, m).rearrange("p (t par j) -> p t par j",
                                          t=NT, par=2)
            nc.vector.tensor_tensor(h3(zs, Lp, m), zv[:, :, 0, :],
                                    zv[:, :, 1, :], ALU.add)

    # --- rh = r * h_sum (in place into hs) ---
    nc.vector.tensor_tensor(hs[:], r[:], hs[:], ALU.mult)

    # --- h_cand = tanh(Uc @ rh + bc) ---
    hcand = P["hc"].tile([128, 2 * Lp], f16, name=f"hcand{tag}", tag="hc")
    _gemm_act(nc, P, "psrc", f"c{tag}", "uc", Wt, "bc", bias, AF.Tanh,
              lambda k, c0, cn: half(hs, Lp, k)[:, c0:c0 + cn],
              lambda m, c0, cn: half(hcand, Lp, m)[:, c0:c0 + cn], Lp)

    # --- zh = z * hc (in place into z, 4D to handle strided jbuf hc) ---
    z4 = z[:].rearrange("p (m t j) -> p m t j", m=2, t=NT)
    for m in range(2):
        nc.vector.tensor_tensor(z4[:, m], z4[:, m], hc4[:, m], ALU.mult)

    # --- zh_sum = zh_even + zh_odd ---
    zhs = P["zhs"].tile([128, 2 * Lp], f16, name=f"zhs{tag}", tag="zhs")
    for m in range(2):
        if wavef:
            for t in range(NT):
                zt = half(z, Lc, m)[:, t * Lct:(t + 1) * Lct]
                nc.vector.tensor_tensor(
                    half(zhs, Lp, m)[:, t * Lpt:(t + 1) * Lpt],
                    zt[:, 0:Lpt], zt[:, Lpt:Lct], ALU.add)
        else:
            zv = half(z, Lc, m).rearrange("p (t par j) -> p t par j",
                                          t=NT, par=2)
            nc.vector.tensor_tensor(h3(zhs, Lp, m), zv[:, :, 0, :],
                                    zv[:, :, 1, :], ALU.add)

    # --- t = (zs - 1) * h_cand, in place; h_new = zh_sum - t (scattered) ---
    nc.vector.scalar_tensor_tensor(hcand[:], zs[:], 1.0, hcand[:],
                                   ALU.subtract, ALU.mult)
    for m in range(2):
        if out_half is not None:
            hp = Lpt // 2
            for t in range(NT):
                zt = half(zhs, Lp, m)[:, t * Lpt:(t + 1) * Lpt].rearrange(
                    "p (j par) -> p j par", par=2)
                ct = half(hcand, Lp, m)[:, t * Lpt:(t + 1) * Lpt].rearrange(
                    "p (j par) -> p j par", par=2)
                for par in range(2):
                    dst = out_half(m)[:, t * Lpt + par * hp:
                                      t * Lpt + (par + 1) * hp]
                    nc.vector.tensor_tensor(dst, zt[:, :, par], ct[:, :, par],
                                            ALU.subtract)
            continue
        if lv == 0:
            src_z, src_c = h3(zhs, Lp, m), h3(hcand, Lp, m)
        else:
            src_z = half(zhs, Lp, m).rearrange("p (t j par) -> p t j par",
                                           t=NT, par=2)
            src_c = half(hcand, Lp, m).rearrange("p (t j par) -> p t j par",
                                             t=NT, par=2)
        nc.vector.tensor_tensor(out_scat(m), src_z, src_c, ALU.subtract)


def _build():
    nc = bacc.Bacc("TRN2", debug=False)

    xT_d = nc.dram_tensor("xT", [H, TPC * NLEAF], f16, kind="ExternalInput")
    wT_d = nc.dram_tensor("wT", [H, H], f16, kind="ExternalInput")
    uzT_d = nc.dram_tensor("uzT", [H, H], f16, kind="ExternalInput")
    urT_d = nc.dram_tensor("urT", [H, H], f16, kind="ExternalInput")
    ucT_d = nc.dram_tensor("ucT", [H, H], f16, kind="ExternalInput")
    bw_d = nc.dram_tensor("bw", [H, 1], f32, kind="ExternalInput")
    bz_d = nc.dram_tensor("bz", [H, 1], f32, kind="ExternalInput")
    br_d = nc.dram_tensor("br", [H, 1], f32, kind="ExternalInput")
    bc_d = nc.dram_tensor("bc", [H, 1], f32, kind="ExternalInput")
    hout_d = nc.dram_tensor("h_out", [H, NCOLS], f16, kind="ExternalOutput")

    def dview(c0, n):
        # DRAM dest [128, 2, n] view of hout cols [c0, c0+n)
        return hout_d.ap()[:, c0:c0 + n].rearrange("(m p) c -> p m c", m=2)

    with tile.TileContext(nc) as tc:
        from contextlib import ExitStack
        with ExitStack() as ctx:
            P = {}
            for nm, bufs in (("const", 1), ("xg", 2), ("h10", 2), ("hl", 2),
                             ("jbuf", 1), ("z", 2), ("hs", 2), ("zs", 2),
                             ("zhs", 2), ("r", 2), ("hc", 2)):
                P[nm] = ctx.enter_context(tc.tile_pool(name=nm, bufs=bufs))
            P["psz"] = ctx.enter_context(tc.tile_pool(name="psz", bufs=2, space="PSUM"))
            P["psrc"] = ctx.enter_context(tc.tile_pool(name="psrc", bufs=2, space="PSUM"))

            cp = P["const"]
            Wt = {}
            for nm, d in (("w", wT_d), ("uz", uzT_d), ("ur", urT_d), ("uc", ucT_d)):
                Wt[nm] = [cp.tile([128, H], f16, name=f"{nm}T{k}") for k in range(2)]
                for k in range(2):
                    nc.sync.dma_start(Wt[nm][k][:], d.ap()[k * 128:(k + 1) * 128, :])
            bias = {}
            for nm, d in (("bw", bw_d), ("bz", bz_d), ("br", br_d), ("bc", bc_d)):
                bias[nm] = [cp.tile([128, 1], f32, name=f"{nm}{m}") for m in range(2)]
                for m in range(2):
                    nc.sync.dma_start(bias[nm][m][:], d.ap()[m * 128:(m + 1) * 128, :])

            # joint buffer: heap nodes 0..126 per tree (levels 0..6)
            jbuf = P["jbuf"].tile([128, 2 * TPC * JN], f16, name="jbuf")
            jv4 = jbuf[:].rearrange("p (m t n) -> p m t n", m=2, t=TPC)

            def emit_leaf(g):
                gt = f"g{g}"
                C = G * NLEAF
                xg = P["xg"].tile([128, 2 * C], f16, name=f"x{gt}", tag="xg")
                for k in range(2):
                    for c0 in range(0, C, 1024):
                        nc.sync.dma_start(
                            xg[:, k * C + c0:k * C + c0 + 1024],
                            xT_d.ap()[k * 128:(k + 1) * 128,
                                      g * C + c0:g * C + c0 + 1024])
                h10 = P["h10"].tile([128, 2 * C], f16, name=f"h10{gt}", tag="h10")
                _gemm_act(nc, P, "psz", f"x{gt}", "w", Wt, "bw", bias, AF.Tanh,
                          lambda k, c0, cn: xg[:, k * C + c0:k * C + c0 + cn],
                          lambda m, c0, cn: h10[:, m * C + c0:m * C + c0 + cn],
                          C)
                nc.sync.dma_start(
                    dview(OFF_LF + g * C, C),
                    h10[:].rearrange("p (m c) -> p m c", m=2))
                return h10

            def emit_lvl(g, lv, hchild):
                gt = f"g{g}"
                Lct = 1 << (lv + 1)        # children per tree (level lv+1 size)
                Lpt = 1 << lv
                hc4 = hchild[:].rearrange("p (m t j) -> p m t j", m=2, t=G)
                if lv == 6:
                    def out_scat(m):
                        blk = jv4[:, m, g * G:(g + 1) * G, 63:127]
                        return blk.rearrange("p t (par j) -> p t j par", par=2)
                    _emit_level(nc, P, f"{gt}l{lv}", G, Lct, hc4, out_scat,
                                Wt, bias, lv)
                    return None
                hnew = P["hl"].tile([128, 2 * G * Lpt], f16,
                                    name=f"h{lv}{gt}", tag=f"h{lv}")

                def out_half(m):
                    return hnew[:, m * G * Lpt:(m + 1) * G * Lpt]
                _emit_level(nc, P, f"{gt}l{lv}", G, Lct, hc4, None,
                            Wt, bias, lv, out_half=out_half)
                off = {9: OFF_L9, 8: OFF_L8, 7: OFF_L7}[lv]
                nc.sync.dma_start(
                    dview(off + g * G * Lpt, G * Lpt),
                    hnew[:].rearrange("p (m c) -> p m c", m=2))
                return hnew

            # wavefront: stage s of group g emitted at tick t = g + s
            gstate = {}
            for t in range(NG + 4):
                for g in range(NG):
                    s = t - g
                    if s < 0 or s > 4:
                        continue
                    if s == 0:
                        gstate[g] = emit_leaf(g)
                    else:
                        gstate[g] = emit_lvl(g, 10 - s, gstate[g])

            # joint levels 5..0 over jbuf
            for lv in range(5, -1, -1):
                Lct = 1 << (lv + 1)
                Lpt = 1 << lv
                hc4 = jv4[:, :, :, Lct - 1:2 * Lct - 1]
                if lv == 0:
                    def out_scat(m):
                        return jv4[:, m, :, 0:1]
                else:
                    def out_scat(m):
                        blk = jv4[:, m, :, Lpt - 1:2 * Lpt - 1]
                        return blk.rearrange("p t (par j) -> p t j par", par=2)
                _emit_level(nc, P, f"j{lv}", TPC, Lct, hc4, out_scat,
                            Wt, bias, lv)

            nc.sync.dma_start(dview(OFF_JB, TPC * JN),
                              jbuf[:].rearrange("p (m c) -> p m c", m=2))

    nc.compile()
    return nc


_NC = None


def _get_nc():
    global _NC
    if _NC is None:
        _NC = _build()
    return _NC


# position perm: parity-major order within a level block
_POS_LEAF = np.concatenate([np.arange(0, NLEAF, 2), np.arange(1, NLEAF, 2)])


def make_in_maps(inputs):
    x = np.asarray(inputs["x"], np.float32)
    W = np.asarray(inputs["W"], np.float32)
    bW = np.asarray(inputs["bW"], np.float32).reshape(H, 1)
    Ur = np.asarray(inputs["Ur"], np.float32)
    br = np.asarray(inputs["br"], np.float32).reshape(H, 1)
    Uc = np.asarray(inputs["Uc"], np.float32)
    bc = np.asarray(inputs["bc"], np.float32).reshape(H, 1)
    Uz = np.asarray(inputs["Uz"], np.float32)
    bz = np.asarray(inputs["bz"], np.float32).reshape(H, 1)
    shared = {
        "wT": np.ascontiguousarray(W.T).astype(np.float16),
        "uzT": np.ascontiguousarray(Uz.T).astype(np.float16),
        "urT": np.ascontiguousarray(Ur.T).astype(np.float16),
        "ucT": np.ascontiguousarray(Uc.T).astype(np.float16),
        "bw": bW, "bz": bz, "br": br, "bc": bc,
    }
    in_maps = []
    for c in range(NCORES):
        # leaf slice, parity-major leaf order: [16, 1024, 256]
        xs = x[c * TPC:(c + 1) * TPC, LEAF0:, :][:, _POS_LEAF, :]
        xTc = np.ascontiguousarray(
            xs.transpose(2, 0, 1).reshape(H, TPC * NLEAF)).astype(np.float16)
        in_maps.append({"xT": xTc, **shared})
    return in_maps


def _out_perm():
    """perm[t, n] -> device column of (tree t, heap node n)."""
    perm = np.empty((TPC, NN), np.int64)
    for n in range(NN):
        lam = (n + 1).bit_length() - 1
        i = n - ((1 << lam) - 1)
        L = 1 << lam
        pos = i if L == 1 else (i & 1) * (L // 2) + (i >> 1)
        for t in range(TPC):
            g, tg = t // G, t % G
            if lam == 10:
                col = OFF_LF + g * (G * 1024) + tg * 1024 + pos
            elif lam == 9:
                col = OFF_L9 + g * (G * 512) + tg * 512 + pos
            elif lam == 8:
                col = OFF_L8 + g * (G * 256) + tg * 256 + pos
            elif lam == 7:
                col = OFF_L7 + g * (G * 128) + tg * 128 + pos
            else:
                col = OFF_JB + t * JN + (L - 1) + pos
            perm[t, n] = col
    return perm


_PERM = _out_perm()


def assemble_out(core_outs):
    out = np.empty((T, NN, H), np.float32)
    for c in range(NCORES):
        buf = core_outs[c]                       # [256, 32752] f16
        g = buf[:, _PERM.ravel()]                # [256, 16*2047]
        out[c * TPC:(c + 1) * TPC] = (
            g.reshape(H, TPC, NN).transpose(1, 2, 0).astype(np.float32))
    return out


def kernel(**inputs):
    nc = _get_nc()
    in_maps = make_in_maps(inputs)
    res = run_bass_kernel_spmd(nc, in_maps, list(range(NCORES)))
    return assemble_out([r["h_out"] for r in res.results])
